# revision 1
# baseline (speedup 1.0000x reference)
"""Trainium2 Bass kernel for nn_Attention_49134425866421.

Dense transformer attention block:
  qkv = x @ W_qkv + b_qkv  -> partial RoPE on q,k -> softmax attention -> out proj.

Shapes (hardcoded): B=4, N=2048, C=768, H=12, D=64, fp32.

Sharding: 8 cores = (batch b in 0..3) x (head-group g in 0..1, 6 heads each).
Each core computes q/k/v projections for its 6 heads, attention, and a partial
output projection (row-parallel over head dims). Host sums the two partials
per batch and adds b_proj.

On-chip layouts (per core):
  xT    [128,6,2048]  x[b]^T, contraction dim c on partitions (c = ko*128+p)
  qT,kT [128,3,2048]  per-head-pair: partition p = 64*(h%2)+d, free (hp, t)
  V     [128,16,6,65] natural: partition = t%128, free (t//128, local head, d)
                      column 64 holds ones -> AV matmul also produces rowsums
  attnT [128,3,2048]  bf16, same layout as qT -> feeds row-parallel proj

RoPE trick: rotate_half is a cross-partition half-swap; done via SBUF->SBUF
DMA of (q * m2s) where m2s = pre-swapped signed sin table, so
q_rope = q*cos + swap(q*m2s). Special (non-rotated) tokens handled by padding
cos=1,sin=0 rows host-side. Softmax without max-subtraction (scores are
N(0,~1); exp never overflows); scale 1/8 folded into the ACT exp call;
rowsum via the ones-column of V'.
"""

import os
import sys

import numpy as np

try:
    import concourse.bass as bass  # noqa: F401
except ImportError:
    sys.path.insert(0, "/opt/trn_rl_repo")

import ml_dtypes

B, N, C, H, D = 4, 2048, 768, 12, 64
HPC = 6          # heads per core
NPAIR = 3        # head pairs per core
P = 128
NT = N // P      # 16 token tiles
TC = 512         # token chunk for matmul free dim
NTC = N // TC    # 4

_NC_CACHE = {}
LAST_RESULTS = None  # BassKernelResults stash for test.py


def _build_nc():
    from contextlib import ExitStack

    import concourse.bass as bass
    import concourse.bacc as bacc
    import concourse.mybir as mybir
    import concourse.tile as tile

    f32 = mybir.dt.float32
    f32r = mybir.dt.float32r
    bf16 = mybir.dt.bfloat16
    EXP = mybir.ActivationFunctionType.Exp

    nc = bacc.Bacc(None, target_bir_lowering=False)

    xT_d = nc.dram_tensor("xT", [C, N], f32r, kind="ExternalInput")
    wqk_d = nc.dram_tensor("w_qk", [P, 6, 768], f32r, kind="ExternalInput")
    wv_d = nc.dram_tensor("w_v", [P, 6, 384], f32r, kind="ExternalInput")
    wp_d = nc.dram_tensor("w_p", [P, 3, 768], bf16, kind="ExternalInput")
    bqk_d = nc.dram_tensor("b_qk", [1, 768], f32r, kind="ExternalInput")
    bv_d = nc.dram_tensor("b_v", [1, 384], f32r, kind="ExternalInput")
    ones_d = nc.dram_tensor("ones", [1, TC], f32r, kind="ExternalInput")
    bqkt_d = nc.dram_tensor("b_qk_t", [P, 6], f32, kind="ExternalInput")
    cos_d = nc.dram_tensor("cos_tab", [P, N], f32, kind="ExternalInput")
    m2s_d = nc.dram_tensor("m2s_tab", [P, N], f32, kind="ExternalInput")
    y_d = nc.dram_tensor("y", [N, C], f32, kind="ExternalOutput")

    with tile.TileContext(nc) as tc, ExitStack() as ctx:
        singles = ctx.enter_context(tc.tile_pool(name="singles", bufs=1))
        mm_ps = ctx.enter_context(tc.tile_pool(name="mm_ps", bufs=2, space="PSUM"))
        att_ps = ctx.enter_context(tc.tile_pool(name="att_ps", bufs=2, space="PSUM"))
        acc_ps = ctx.enter_context(tc.tile_pool(name="acc_ps", bufs=1, space="PSUM"))
        rope_tmp = ctx.enter_context(tc.tile_pool(name="rope_tmp", bufs=2))
        pt_pool = ctx.enter_context(tc.tile_pool(name="pt", bufs=3))
        rb_pool = ctx.enter_context(tc.tile_pool(name="rb", bufs=2))
        y_pool = ctx.enter_context(tc.tile_pool(name="yout", bufs=2))

        # ---- static SBUF tensors ----
        xT = singles.tile([P, 6, N], f32r)
        wqk = singles.tile([P, 6, 768], f32r)
        wv = singles.tile([P, 6, 384], f32r)
        wp = singles.tile([P, 3, 768], bf16)
        bqk = singles.tile([1, 768], f32r)
        bv = singles.tile([1, 384], f32r)
        cosT = singles.tile([P, N], f32)
        m2sT = singles.tile([P, N], f32)
        ones = singles.tile([1, TC], f32r)
        bqkt = singles.tile([P, 6], f32)
        qT = singles.tile([P, NPAIR, N], f32r)
        kT = singles.tile([P, NPAIR, N], f32r)
        Vt = singles.tile([P, NT, HPC, D + 1], bf16)
        attnT = singles.tile([P, NPAIR, N], bf16)

        xT_r = xT_d.rearrange("(ko p) t -> p ko t", p=P)
        for ko in range(6):
            nc.sync.dma_start(xT[:, ko, :], xT_r[:, ko, :])
        nc.scalar.dma_start(wqk[:], wqk_d[:])
        nc.sync.dma_start(wv[:], wv_d[:])
        nc.scalar.dma_start(bqk[:], bqk_d[:])
        nc.sync.dma_start(bv[:], bv_d[:])
        nc.sync.dma_start(ones[:], ones_d[:])
        nc.sync.dma_start(bqkt[:], bqkt_d[:])
        nc.scalar.dma_start(cosT[:], cos_d[:])
        nc.scalar.dma_start(m2sT[:], m2s_d[:])
        nc.gpsimd.memset(Vt[:], 1.0)

        def emit_qk(hp):
            for tcu in range(NTC):
                tsl = slice(tcu * TC, (tcu + 1) * TC)
                for mt in (3 + hp, hp):  # k pair first, then q pair
                    dst = qT if mt < 3 else kT
                    ps = mm_ps.tile([P, TC], f32, tag="mm")
                    for ko in range(6):
                        nc.tensor.matmul(
                            ps,
                            lhsT=wqk[:, ko, mt * P : (mt + 1) * P],
                            rhs=xT[:, ko, tsl],
                            start=(ko == 0),
                            stop=(ko == 5),
                        )
                    # bias add on DVE, then rope: dst = pb*cos + swap(pb*m2s)
                    pb = rope_tmp.tile([P, TC], f32, tag="pb")
                    qs = rope_tmp.tile([P, TC], f32, tag="qs")
                    qsw = rope_tmp.tile([P, TC], f32, tag="qsw")
                    nc.vector.tensor_scalar_add(
                        out=pb[:], in0=ps[:], scalar1=bqkt[:, mt : mt + 1]
                    )
                    nc.vector.tensor_mul(out=qs[:], in0=pb[:], in1=m2sT[:, tsl])
                    nc.vector.tensor_mul(
                        out=dst[:, hp, tsl], in0=pb[:], in1=cosT[:, tsl]
                    )
                    for blk in range(4):
                        sp = [1, 0, 3, 2][blk] * 32
                        nc.sync.dma_start(
                            out=qsw[blk * 32 : blk * 32 + 32, :],
                            in_=qs[sp : sp + 32, :],
                        )
                    nc.vector.tensor_add(
                        out=dst[:, hp, tsl], in0=dst[:, hp, tsl], in1=qsw[:]
                    )

        emit_qk(0)
        nc.sync.dma_start(wp[:], wp_d[:])

        # ---- V projection (natural layout), all 6 heads; emitted per
        # token-tile, interleaved into the first attention pass ----
        def emit_v(tt):
            ps = mm_ps.tile([P, TC], f32, tag="mm")
            vps = ps[:, :384]
            for ko in range(6):
                nc.tensor.matmul(
                    vps,
                    lhsT=xT[:, ko, tt * P : (tt + 1) * P],
                    rhs=wv[:, ko, :],
                    start=(ko == 0),
                    stop=False,
                )
            nc.tensor.matmul(
                vps, lhsT=ones[:, :P], rhs=bv[:], start=False, stop=True
            )
            nc.vector.tensor_copy(
                out=Vt[:, tt, :, :D],
                in_=vps.rearrange("p (h d) -> p h d", h=HPC),
            )

        # ---- per head-pair: q/k projection + RoPE, then attention ----
        for hp in range(NPAIR):
            if hp > 0:
                emit_qk(hp)

            # attention for the two heads of this pair
            for ic in range(NTC):
                isl = slice(ic * TC, (ic + 1) * TC)
                accA = acc_ps.tile([D + 1, TC], f32, tag="accA")
                accB = acc_ps.tile([D + 1, TC], f32, tag="accB")
                for jt in range(NT):
                    if hp == 0 and ic == 0:
                        emit_v(jt)
                    st = att_ps.tile([P, 2 * TC], f32, tag="st")
                    nc.tensor.matmul(
                        st[:, :TC],
                        lhsT=kT[:D, hp, jt * P : (jt + 1) * P],
                        rhs=qT[:D, hp, isl],
                        start=True,
                        stop=True,
                        tile_position=(0, 0),
                    )
                    nc.tensor.matmul(
                        st[:, TC:],
                        lhsT=kT[D:, hp, jt * P : (jt + 1) * P],
                        rhs=qT[D:, hp, isl],
                        start=True,
                        stop=True,
                        tile_position=(64, 0),
                    )
                    pt = pt_pool.tile([P, 2 * TC], bf16, tag="pt")
                    if os.environ.get("ABLATE") == "exp":
                        nc.scalar.activation(pt[:, :8], st[:, :8], EXP, scale=0.125)
                        nc.scalar.activation(pt[:, 8:], st[:, 8:], EXP, scale=0.125) if False else None
                    else:
                        nc.scalar.activation(pt[:], st[:], EXP, scale=0.125)
                    nc.tensor.matmul(
                        accA,
                        lhsT=Vt[:, jt, 2 * hp, :],
                        rhs=pt[:, :TC],
                        start=(jt == 0),
                        stop=(jt == NT - 1),
                    )
                    nc.tensor.matmul(
                        accB,
                        lhsT=Vt[:, jt, 2 * hp + 1, :],
                        rhs=pt[:, TC:],
                        start=(jt == 0),
                        stop=(jt == NT - 1),
                    )
                # evacuate PSUM accs to SBUF immediately so the banks free up
                # for the next i-chunk; rescale then runs off the PE critical
                # path entirely.
                accs = []
                for half, acc in ((0, accA), (1, accB)):
                    asb = rb_pool.tile([D + 1, TC], f32, tag="asb")
                    nc.vector.tensor_copy(out=asb[:], in_=acc[:])
                    accs.append(asb)
                for half, asb in ((0, accs[0]), (1, accs[1])):
                    rec = rb_pool.tile([1, TC], f32, tag="rec")
                    rbc = rb_pool.tile([D, TC], f32, tag="rbc")
                    nc.vector.reciprocal(out=rec[:], in_=asb[D : D + 1, :])
                    nc.gpsimd.partition_broadcast(rbc[:], rec[:], channels=D)
                    nc.vector.tensor_mul(
                        out=attnT[half * D : (half + 1) * D, hp, isl],
                        in0=asb[:D, :],
                        in1=rbc[:],
                    )

        # ---- output projection (row-parallel partial) ----
        for tt in range(NT):
            for ch in range(2):
                ps = mm_ps.tile([P, TC], f32, tag="mm")
                yps = ps[:, :384]
                for ko in range(3):
                    nc.tensor.matmul(
                        yps,
                        lhsT=attnT[:, ko, tt * P : (tt + 1) * P],
                        rhs=wp[:, ko, ch * 384 : (ch + 1) * 384],
                        start=(ko == 0),
                        stop=(ko == 2),
                    )
                yt = y_pool.tile([P, 384], f32, tag="yt")
                nc.vector.tensor_copy(out=yt[:], in_=yps)
                nc.sync.dma_start(
                    out=y_d[tt * P : (tt + 1) * P, ch * 384 : (ch + 1) * 384],
                    in_=yt[:],
                )

    nc.finalize()
    return nc


def _host_inputs(x, rope_cos, rope_sin, W_qkv, b_qkv, W_proj, b_proj, num_special):
    ns = int(num_special)
    cos_pad = np.ones((N, D), np.float32)
    sin_pad = np.zeros((N, D), np.float32)
    cos_pad[ns:] = rope_cos
    sin_pad[ns:] = rope_sin
    # m2s[t, d] = +sin[t, d+32] (d<32) else -sin[t, d-32]
    m2s = np.empty_like(sin_pad)
    m2s[:, : D // 2] = sin_pad[:, D // 2 :]
    m2s[:, D // 2 :] = -sin_pad[:, : D // 2]
    cos_tab = np.tile(np.ascontiguousarray(cos_pad.T), (2, 1))
    m2s_tab = np.tile(np.ascontiguousarray(m2s.T), (2, 1))

    in_maps = []
    for core in range(8):
        b, g = core // 2, core % 2
        hs = list(range(HPC * g, HPC * g + HPC))
        cols_qk = []
        for mt in range(6):
            s, hp = (0, mt) if mt < 3 else (1, mt - 3)
            for half in range(2):
                h = hs[2 * hp + half]
                cols_qk.extend(s * 768 + h * 64 + d for d in range(D))
        cols_qk = np.array(cols_qk)
        cols_v = np.array([2 * 768 + hs[i // 64] * 64 + (i % 64) for i in range(384)])
        rows_p = np.array(
            [hs[2 * ko + half] * 64 + d
             for ko in range(3) for half in range(2) for d in range(D)]
        )
        in_maps.append({
            "xT": np.ascontiguousarray(x[b].T),
            "w_qk": np.ascontiguousarray(
                W_qkv[:, cols_qk].reshape(6, P, 768).transpose(1, 0, 2)),
            "w_v": np.ascontiguousarray(
                W_qkv[:, cols_v].reshape(6, P, 384).transpose(1, 0, 2)),
            "w_p": np.ascontiguousarray(
                W_proj[rows_p].reshape(3, P, 768).transpose(1, 0, 2)
            ).astype(ml_dtypes.bfloat16),
            "b_qk": np.ascontiguousarray(b_qkv[cols_qk].reshape(1, 768)),
            "b_qk_t": np.ascontiguousarray(
                b_qkv[cols_qk].reshape(6, P).T),
            "b_v": np.ascontiguousarray(b_qkv[cols_v].reshape(1, 384)),
            "ones": np.ones((1, TC), np.float32),
            "cos_tab": cos_tab,
            "m2s_tab": m2s_tab,
        })
    return in_maps


def kernel(x, rope_cos, rope_sin, W_qkv, b_qkv, W_proj, b_proj, num_special):
    global LAST_RESULTS
    from concourse.bass_utils import run_bass_kernel_spmd

    x = np.asarray(x, np.float32)
    if "nc" not in _NC_CACHE:
        _NC_CACHE["nc"] = _build_nc()
    nc = _NC_CACHE["nc"]

    in_maps = _host_inputs(
        x, np.asarray(rope_cos, np.float32), np.asarray(rope_sin, np.float32),
        np.asarray(W_qkv, np.float32), np.asarray(b_qkv, np.float32),
        np.asarray(W_proj, np.float32), np.asarray(b_proj, np.float32), num_special,
    )
    trace = bool(int(os.environ.get("KERNEL_TRACE", "0")))
    res = run_bass_kernel_spmd(nc, in_maps, core_ids=list(range(8)), trace=trace)
    LAST_RESULTS = res

    bp = np.asarray(b_proj, np.float32)
    out = np.empty((B, N, C), np.float32)
    for b in range(B):
        out[b] = res.results[2 * b]["y"] + res.results[2 * b + 1]["y"] + bp
    return out



# revision 41
# speedup vs baseline: 1.2786x; 1.2786x over previous
"""Trainium2 Bass kernel for nn_Attention_49134425866421.

Dense transformer attention block:
  qkv = x @ W_qkv + b_qkv  -> partial RoPE on q,k -> softmax attention -> out proj.

Shapes (hardcoded): B=4, N=2048, C=768, H=12, D=64, fp32 in/out.

Sharding: 8 cores = (batch b in 0..3) x (head-group g in 0..1, 6 heads each).
Each core computes q/k/v projections for its 6 heads, attention, and a partial
output projection (row-parallel over head dims). Host sums the two partials
per batch and adds the combined bias (b_proj + b_v @ W_proj — the V bias
commutes through softmax averaging, so it is folded into the output bias
host-side and V is projected without bias).

Schedule: the ACT engine's exp stream (192 x [128,1024] activations,
~200us) is the roofline. Everything else is interleaved so ACT never
starves:
  - preamble: q(tcu0)+k(all) projection chunks overlap the xT DMA stream
  - attention (hp, ic): per jt: scores (2 matmuls) -> exp -> flipped AV,
    with AV trailing scores by 2 jt so PSUM-acc reuse stalls never block
    the score->exp stream
  - qkv projection chunks for later head-pairs, V projection pair-chunks,
    and output projection chunks are placed in per-jt "extra" slots.

Flipped AV: out[q(128 part), d] += pt[k, qblock].T @ V[k, d+1] with a ones
column in V producing the softmax rowsum for free; cost 65 free-rows per
accumulation step vs 512 for the [d, q] orientation.  The [q, hd] result is
rescaled (per-partition reciprocal scalars) to bf16 and DMA-XBAR-transposed
back into the [hd, q] layout the output projection needs.

On-chip layouts (per core):
  xT    [128,6,2048] bf16  x[b]^T, contraction c on partitions (c=ko*128+p)
  qT,kT [128,3,2048] f32r  per head-pair: partition p = 64*(h%2)+d
  Vt    [128,16,6,65] bf16 partition = t%128, free (t//128, head, d); col 64 = 1
  attnT [128,3,2048] bf16  rescaled attention output, same layout as qT

RoPE trick: rotate_half via SBUF->SBUF DMA of (q * m2s), m2s = pre-swapped
signed sin table; special tokens handled by cos=1,sin=0 padding host-side.
Softmax without max-subtraction (scores ~N(0,1)); 1/8 scale folded into the
exp activation.
"""

import os
import sys

import numpy as np

try:
    import concourse.bass as bass  # noqa: F401
except ImportError:
    sys.path.insert(0, "/opt/trn_rl_repo")

import ml_dtypes

B, N, C, H, D = 4, 2048, 768, 12, 64
HPC = 6          # heads per core
NPAIR = 3        # head pairs per core
P = 128
NT = N // P      # 16 token tiles
TC = 512         # token chunk for matmul free dim
NTC = N // TC    # 4

_NC_CACHE = {}
LAST_RESULTS = None  # BassKernelResults stash for test.py


def _build_nc():
    from contextlib import ExitStack

    import concourse.bass as bass
    import concourse.bacc as bacc
    import concourse.mybir as mybir
    import concourse.tile as tile

    f32 = mybir.dt.float32
    f32r = mybir.dt.float32r
    bf16 = mybir.dt.bfloat16
    EXP = mybir.ActivationFunctionType.Exp

    nc = bacc.Bacc(None, target_bir_lowering=False)

    xT_d = nc.dram_tensor("xT", [C, N], bf16, kind="ExternalInput")
    wqk_d = nc.dram_tensor("w_qk", [P, 6, 768], bf16, kind="ExternalInput")
    wv_d = nc.dram_tensor("w_v", [P, 6, 384], bf16, kind="ExternalInput")
    wp_d = nc.dram_tensor("w_p", [P, 3, 768], bf16, kind="ExternalInput")
    brow_d = nc.dram_tensor("b_row", [1, 768], bf16, kind="ExternalInput")
    ones_d = nc.dram_tensor("ones_row", [1, TC], bf16, kind="ExternalInput")
    perm_d = nc.dram_tensor("perm", [P, P], bf16, kind="ExternalInput")
    cos_d = nc.dram_tensor("cos_tab", [P, N], bf16, kind="ExternalInput")
    m2s_d = nc.dram_tensor("m2s_tab", [P, N], bf16, kind="ExternalInput")
    y_d = nc.dram_tensor("y", [N, C], bf16, kind="ExternalOutput")

    with tile.TileContext(nc) as tc, ExitStack() as ctx:
        singles = ctx.enter_context(tc.tile_pool(name="singles", bufs=1))
        mm_ps = ctx.enter_context(tc.tile_pool(name="mm_ps", bufs=2, space="PSUM"))
        att_ps = ctx.enter_context(tc.tile_pool(name="att_ps", bufs=2, space="PSUM"))
        acc_ps = ctx.enter_context(tc.tile_pool(name="acc_ps", bufs=1, space="PSUM"))
        rope_tmp = ctx.enter_context(tc.tile_pool(name="rope_tmp", bufs=2))
        pt_pool = ctx.enter_context(tc.tile_pool(name="pt", bufs=6))
        rb_pool = ctx.enter_context(tc.tile_pool(name="rb", bufs=2))
        y_pool = ctx.enter_context(tc.tile_pool(name="yout", bufs=2))

        # ---- static SBUF tensors ----
        xT = singles.tile([P, 6, N], bf16)
        wqk = singles.tile([P, 6, 768], bf16)
        wv = singles.tile([P, 6, 384], bf16)
        wp = singles.tile([P, 3, 768], bf16)
        cosT = singles.tile([P, N], bf16)
        m2sT = singles.tile([P, N], bf16)
        brow = singles.tile([1, 768], bf16)
        ones_row = singles.tile([1, TC], bf16)
        perm = singles.tile([P, P], bf16)
        qT = singles.tile([P, NPAIR, N], f32r)
        kT = singles.tile([P, NPAIR, N], f32r)
        Vt = singles.tile([P, NT, HPC, D + 1], bf16)
        attnT = singles.tile([P, NPAIR, N], bf16)

        # DMA order matters: everything the first q/k chunks + ropes need
        # (wqk cols for hp0, bias, rope tables) lands before the xT stream;
        # the rest follows it.
        nc.sync.dma_start(wqk[:, :, : 4 * P], wqk_d[:, :, : 4 * P])
        nc.sync.dma_start(brow[:], brow_d[:])
        nc.sync.dma_start(ones_row[:], ones_d[:])
        nc.sync.dma_start(perm[:], perm_d[:])
        nc.sync.dma_start(m2sT[:], m2s_d[:])
        # xT streams token-block-major: the tcu0 projections (and their
        # ropes) start after the first quarter instead of the full tensor.
        # cos/wv interleave where their first consumers need them.
        xT_r = xT_d.rearrange("(ko p) t -> p ko t", p=P)
        for t in range(NTC):
            tsl = slice(t * TC, (t + 1) * TC)
            for ko in range(6):
                nc.sync.dma_start(xT[:, ko, tsl], xT_r[:, ko, tsl])
            if t == 0:
                nc.sync.dma_start(cosT[:], cos_d[:])
            elif t == 1:
                nc.sync.dma_start(wv[:], wv_d[:])
        nc.sync.dma_start(wqk[:, :, 4 * P :], wqk_d[:, :, 4 * P :])
        nc.sync.dma_start(wp[:], wp_d[:])
        nc.gpsimd.memset(Vt[:], 1.0)

        # warm the exp table on ACT before the score stream needs it
        warm = singles.tile([P, 8], f32)
        nc.scalar.activation(warm[:1, :8], brow[:, :8], EXP, scale=0.125)
        # warm the PE clock: back-to-back dummy matmuls from the tiny perm
        # tile keep the p-state ramp alive through the DMA-paced preamble.
        wps = mm_ps.tile([P, TC], f32, tag="mm")
        for _ in range(40):
            nc.tensor.matmul(
                wps[:, :P], lhsT=perm[:], rhs=perm[:], start=True, stop=True
            )

        def emit_qk_chunk(hp, tcu, which):
            """q or k projection + rope for head pair hp, token chunk tcu.
            rotate_half's partition swap (p <-> p^32) runs on the PE as a
            permutation matmul into PSUM -- no DMA involved."""
            mt = hp if which == "q" else 3 + hp
            dst = qT if which == "q" else kT
            tsl = slice(tcu * TC, (tcu + 1) * TC)
            ps = mm_ps.tile([P, TC], f32, tag="mm")
            for ko in range(6):
                nc.tensor.matmul(
                    ps,
                    lhsT=wqk[:, ko, mt * P : (mt + 1) * P],
                    rhs=xT[:, ko, tsl],
                    start=(ko == 0),
                    stop=False,
                )
            # bias folded in as a rank-1 accumulation step (ones row x bias)
            nc.tensor.matmul(
                ps,
                lhsT=brow[:, mt * P : (mt + 1) * P],
                rhs=ones_row[:],
                start=False,
                stop=True,
            )
            # rope: dst = ps*cos + perm.T @ (ps*m2s); the swap matmul
            # overwrites ps in place once both muls have read it, so each
            # chunk occupies a single PSUM bank.
            qs = rope_tmp.tile([P, TC], bf16, tag="qs")
            nc.vector.tensor_mul(out=qs[:], in0=ps[:], in1=m2sT[:, tsl])
            nc.vector.tensor_mul(out=dst[:, hp, tsl], in0=ps[:], in1=cosT[:, tsl])
            nc.tensor.matmul(ps, lhsT=perm[:], rhs=qs[:], start=True, stop=True)
            nc.vector.tensor_add(
                out=dst[:, hp, tsl], in0=dst[:, hp, tsl], in1=ps[:]
            )

        def emit_qk_pair(hp, tcu):
            """q AND k projection + rope for head pair hp, chunk tcu."""
            emit_qk_chunk(hp, tcu, "k")
            emit_qk_chunk(hp, tcu, "q")

        def emit_v_pair(jp, hp):
            """V projection for head pair hp, token tiles 2jp, 2jp+1."""
            ps = mm_ps.tile([P, TC], f32, tag="mm")
            for jtl in (0, 1):
                jt = 2 * jp + jtl
                vp = ps[:, jtl * P : (jtl + 1) * P]
                for ko in range(6):
                    nc.tensor.matmul(
                        vp,
                        lhsT=xT[:, ko, jt * P : (jt + 1) * P],
                        rhs=wv[:, ko, hp * P : (hp + 1) * P],
                        start=(ko == 0),
                        stop=(ko == 5),
                    )
            nc.vector.tensor_copy(
                out=Vt[:, 2 * jp : 2 * jp + 2, 2 * hp : 2 * hp + 2, :D],
                in_=ps[:, : 2 * P].rearrange("p (j h d) -> p j h d", j=2, h=2),
            )

        def emit_proj(tt, copy_eng=None):
            """output projection for token tile tt (all 768 channels)."""
            copy_eng = copy_eng or nc.vector
            yt = y_pool.tile([P, 768], bf16, tag="yt")
            for ch in range(2):
                ps = mm_ps.tile([P, TC], f32, tag="mm")
                yps = ps[:, :384]
                for ko in range(3):
                    nc.tensor.matmul(
                        yps,
                        lhsT=attnT[:, ko, tt * P : (tt + 1) * P],
                        rhs=wp[:, ko, ch * 384 : (ch + 1) * 384],
                        start=(ko == 0),
                        stop=(ko == 2),
                    )
                if copy_eng is nc.scalar:
                    nc.scalar.copy(out=yt[:, ch * 384 : (ch + 1) * 384], in_=yps)
                else:
                    nc.vector.tensor_copy(
                        out=yt[:, ch * 384 : (ch + 1) * 384], in_=yps
                    )
            nc.sync.dma_start(out=y_d[tt * P : (tt + 1) * P, :], in_=yt[:])

        def attention(hp, ic, extras, fuse_proj=False):
            """Scores+exp+AV for head pair hp over q-chunk ic, with extra
            work thunks interleaved at given jt slots."""
            isl = slice(ic * TC, (ic + 1) * TC)
            # padded to a full 2KB PSUM bank so tiles stay bank-aligned
            accA = acc_ps.tile([P, NTC, P], f32, tag="accA")
            accB = acc_ps.tile([P, NTC, P], f32, tag="accB")
            accs = [accA, accB]
            pts = {}

            def av(jt):
                # start=True zeroes the whole 2KB PSUM region, so only the
                # first group (qb=0) of each acc bank may set it; the other
                # qb groups accumulate onto the freshly zeroed region.
                pt = pts.pop(jt)
                for h2 in (0, 1):
                    for qb in range(NTC):
                        nc.tensor.matmul(
                            accs[h2][:, qb, : D + 1],
                            lhsT=pt[:, h2 * TC + qb * P : h2 * TC + (qb + 1) * P],
                            rhs=Vt[:, jt, 2 * hp + h2, :],
                            start=(jt == 0 and qb == 0),
                            stop=(jt == NT - 1),
                            skip_group_check=True,
                        )

            for jt in range(NT):
                for th in extras.get(jt, ()):
                    th()
                with tc.high_priority():
                    st = att_ps.tile([P, 2 * TC], f32, tag="st")
                    nc.tensor.matmul(
                        st[:, :TC],
                        lhsT=kT[:D, hp, jt * P : (jt + 1) * P],
                        rhs=qT[:D, hp, isl],
                        start=True,
                        stop=True,
                        tile_position=(0, 0),
                    )
                    nc.tensor.matmul(
                        st[:, TC:],
                        lhsT=kT[D:, hp, jt * P : (jt + 1) * P],
                        rhs=qT[D:, hp, isl],
                        start=True,
                        stop=True,
                        tile_position=(64, 0),
                    )
                    pt = pt_pool.tile([P, 2 * TC], bf16, tag="pt")
                    nc.scalar.activation(pt[:], st[:], EXP, scale=0.125)
                    pts[jt] = pt
                    if jt >= 2:
                        av(jt - 2)
            with tc.high_priority():
                av(NT - 2)
                av(NT - 1)

            # rescale (per-partition 1/rowsum) + XBAR transpose to attnT
            with tc.high_priority():
                recs = []
                for h2 in (0, 1):
                    rec = rb_pool.tile([P, NTC], f32, tag=f"rec{h2}")
                    nc.vector.reciprocal(out=rec[:], in_=accs[h2][:, :, D])
                    recs.append(rec)
                for qb in range(NTC):
                    ao = rb_pool.tile([P, P], bf16, tag=f"ao{qb}")
                    for h2 in (0, 1):
                        nc.vector.tensor_scalar_mul(
                            out=ao[:, h2 * D : (h2 + 1) * D],
                            in0=accs[h2][:, qb, :D],
                            scalar1=recs[h2][:, qb : qb + 1],
                        )
                    nc.sync.dma_start_transpose(
                        out=attnT[:, hp, ic * TC + qb * P : ic * TC + (qb + 1) * P],
                        in_=ao[:],
                    )
                    if fuse_proj:
                        emit_proj(ic * NTC + qb, copy_eng=nc.scalar)

        # ---- preamble: q+k(tcu0), k(tcu1-3) for hp0, first V pairs.
        # The V pairs sit between k1 and k2 so their DVE copies beat the
        # later rope ops (priority = emission order) -- AV(jt0/jt2) needs
        # them to release pt buffers for the exp stream. ----
        emit_qk_pair(0, 0)
        emit_qk_chunk(0, 1, "k")
        emit_v_pair(0, 0)
        emit_v_pair(1, 0)
        emit_qk_chunk(0, 2, "k")
        emit_qk_chunk(0, 3, "k")

        # ---- extra-work schedule: (hp, ic) -> {jt: [thunks]} ----
        def v_slots(hp, start_jp=0):
            return {
                2 * jp: [lambda jp=jp, hp=hp: emit_v_pair(jp, hp)]
                for jp in range(start_jp, 8)
            }

        def qk1(hp, tcu, which, jt):
            return {jt: [lambda: emit_qk_chunk(hp, tcu, which)]}

        def qk2(hp, tcus, jts):
            return {
                jt: [lambda tcu=tcu: emit_qk_pair(hp, tcu)]
                for tcu, jt in zip(tcus, jts)
            }

        def proj_slots(ic):
            return {
                4 * t + 1: [lambda tt=4 * ic + t: emit_proj(tt)]
                for t in range(4)
            }

        def merge(*ds):
            out = {}
            for d in ds:
                for k, v in d.items():
                    out.setdefault(k, []).extend(v)
            return out

        schedule = {
            (0, 0): merge(v_slots(0, start_jp=2), qk1(0, 1, "q", 3)),
            (0, 1): merge(v_slots(1), qk1(0, 2, "q", 3)),
            (0, 2): merge(qk1(0, 3, "q", 1), qk2(1, (0, 1), (4, 9))),
            (0, 3): qk2(1, (2, 3), (1, 6)),
            (1, 0): v_slots(2),
            (1, 1): qk2(2, (0, 1), (1, 6)),
            (1, 2): qk2(2, (2, 3), (1, 6)),
            (1, 3): {},
            (2, 0): {},
            (2, 1): proj_slots(0),
            (2, 2): proj_slots(1),
            (2, 3): proj_slots(2),
        }

        for hp in range(NPAIR):
            for ic in range(NTC):
                attention(hp, ic, schedule[(hp, ic)],
                          fuse_proj=(hp == 2 and ic == 3))

    nc.finalize()
    return nc


def _host_inputs(x, rope_cos, rope_sin, W_qkv, b_qkv, W_proj, b_proj, num_special):
    bf16 = ml_dtypes.bfloat16
    ns = int(num_special)
    cos_pad = np.ones((N, D), np.float32)
    sin_pad = np.zeros((N, D), np.float32)
    cos_pad[ns:] = rope_cos
    sin_pad[ns:] = rope_sin
    # m2s[t, d] = +sin[t, d+32] (d<32) else -sin[t, d-32]
    m2s = np.empty_like(sin_pad)
    m2s[:, : D // 2] = sin_pad[:, D // 2 :]
    m2s[:, D // 2 :] = -sin_pad[:, : D // 2]
    cos_tab = np.tile(np.ascontiguousarray(cos_pad.T), (2, 1)).astype(bf16)
    m2s_tab = np.tile(np.ascontiguousarray(m2s.T), (2, 1)).astype(bf16)

    in_maps = []
    for core in range(8):
        b, g = core // 2, core % 2
        hs = list(range(HPC * g, HPC * g + HPC))
        cols_qk = []
        for mt in range(6):
            s, hp = (0, mt) if mt < 3 else (1, mt - 3)
            for half in range(2):
                h = hs[2 * hp + half]
                cols_qk.extend(s * 768 + h * 64 + d for d in range(D))
        cols_qk = np.array(cols_qk)
        cols_v = np.array([2 * 768 + hs[i // 64] * 64 + (i % 64) for i in range(384)])
        rows_p = np.array(
            [hs[2 * ko + half] * 64 + d
             for ko in range(3) for half in range(2) for d in range(D)]
        )
        perm_mat = np.zeros((P, P), np.float32)
        perm_mat[np.arange(P), np.arange(P) ^ 32] = 1.0
        in_maps.append({
            "xT": np.ascontiguousarray(x[b].T).astype(bf16),
            "perm": perm_mat.astype(bf16),
            "w_qk": np.ascontiguousarray(
                W_qkv[:, cols_qk].reshape(6, P, 768).transpose(1, 0, 2)
            ).astype(bf16),
            "w_v": np.ascontiguousarray(
                W_qkv[:, cols_v].reshape(6, P, 384).transpose(1, 0, 2)
            ).astype(bf16),
            "w_p": np.ascontiguousarray(
                W_proj[rows_p].reshape(3, P, 768).transpose(1, 0, 2)
            ).astype(bf16),
            "b_row": np.ascontiguousarray(b_qkv[cols_qk].reshape(1, 768)).astype(bf16),
            "ones_row": np.ones((1, TC), np.float32).astype(bf16),
            "cos_tab": cos_tab,
            "m2s_tab": m2s_tab,
        })
    return in_maps


def kernel(x, rope_cos, rope_sin, W_qkv, b_qkv, W_proj, b_proj, num_special):
    global LAST_RESULTS
    from concourse.bass_utils import run_bass_kernel_spmd

    x = np.asarray(x, np.float32)
    if "nc" not in _NC_CACHE:
        _NC_CACHE["nc"] = _build_nc()
    nc = _NC_CACHE["nc"]

    in_maps = _host_inputs(
        x, np.asarray(rope_cos, np.float32), np.asarray(rope_sin, np.float32),
        np.asarray(W_qkv, np.float32), np.asarray(b_qkv, np.float32),
        np.asarray(W_proj, np.float32), np.asarray(b_proj, np.float32), num_special,
    )
    trace = bool(int(os.environ.get("KERNEL_TRACE", "0")))
    res = run_bass_kernel_spmd(nc, in_maps, core_ids=list(range(8)), trace=trace)
    LAST_RESULTS = res

    # V bias commutes through the softmax average: fold b_v @ W_proj into the
    # output bias, added once per batch on the host.
    bq = np.asarray(b_qkv, np.float64)
    bp = (bq[2 * C :] @ np.asarray(W_proj, np.float64)
          + np.asarray(b_proj, np.float64)).astype(np.float32)
    out = np.empty((B, N, C), np.float32)
    for b in range(B):
        out[b] = (res.results[2 * b]["y"].astype(np.float32)
                  + res.results[2 * b + 1]["y"].astype(np.float32) + bp)
    return out
